# revision 55
# baseline (speedup 1.0000x reference)
"""BiGraphSAGEDecoder Trainium2 kernel (v2: fp16 + reassociated matmul chain).

Sharding: 8 cores = 4 batches x {up-path, down-path}. One SPMD bass program;
the up/down asymmetry is handled purely by data (down cores receive host-
transposed adjacency / adjacency-weight / inv-degree matrices).

Key restructurings vs the reference math:
  * associativity:  inv @ (prod^T @ h) @ W  ->  inv @ (prod^T @ (h @ W)),
    so both N x N matmuls run at width 256 instead of width din (768).
  * all feature maps are kept FEATURE-MAJOR (hT: [feature part, node free]);
    column L2 norms via a ones-vector matmul on the PE. No hidden-state
    transposes anywhere.
  * layer 2 computes only the two drug rows after a2 = prod^T @ g2
    (b restricted to 2 rows of inv).
  * fp16 storage end-to-end (PSUM accumulation fp32); host pre-converts.

Per layer (per core, its path), with prod = adj . Wadj' (mask baked on host):
  g    = h @ Wc                      (PE, lhsT = hT tiles)          [N, 256]
  a^T  = (prod^T @ g)^T              (PE, k-outer, 8 psum banks)    [256, N]
  a_sb = transpose(a^T)              (PE transposes)                [N, 256]
  b^T  = (inv @ a)^T                 (PE, lhsT = a_sb, rhs = invT)  [256, N]
  gb^T = (h @ Wb_half)^T             (PE, lhsT = Wb, rhs = hT)      [128, N]
  stage [384, N] = [b^T ; gb^T] -> 2-rank AllGather -> catT [768, N]
  hT'  = lrelu(catT / max(||catT||_col, eps))   (DVE + ones-matmul norms)
Head: bilinear form on PE in fp32 (tiny).
"""

import os
import sys
import types
import contextlib

sys.path.insert(0, "/opt/trn_rl_repo")

import numpy as np

import concourse.bass as bass
import concourse.tile as tile
from concourse import mybir, bacc
from concourse.mybir import AxisListType
from concourse.masks import make_identity
from concourse.bass_utils import run_bass_kernel_spmd

FP = mybir.dt.float32
F16 = mybir.dt.float16
BF = mybir.dt.bfloat16
AF = mybir.ActivationFunctionType
ALU = mybir.AluOpType

# ---------------------------------------------------------------------------
# Environment patches (required for this container's toolchain)
# ---------------------------------------------------------------------------


def install_ntff_shim():
    """antenv.axon_hooks is absent in this image; provide it so trace=True
    profiling works (used by test.py, harmless otherwise)."""
    try:
        import antenv.axon_hooks  # noqa: F401
        return
    except ImportError:
        pass
    try:
        import antenv
    except ImportError:
        return
    mod = types.ModuleType("antenv.axon_hooks")
    _holder = {"hook": None}
    mod.set_axon_ntff_profile_hook = lambda h: _holder.__setitem__("hook", h)
    mod.get_axon_ntff_profile_hook = lambda: _holder["hook"]
    sys.modules["antenv.axon_hooks"] = mod
    antenv.axon_hooks = mod
    try:
        from trn_agent_boot.trn_boot import _ntff_profile_via_ctypes

        hook = _ntff_profile_via_ctypes("/opt/axon/libaxon_pjrt.so")
        if hook is not None:
            mod.set_axon_ntff_profile_hook(hook)
    except Exception:
        pass


install_ntff_shim()

if os.environ.get("KGSD_LDW_OPT", "0") != "0":
    # let walrus dedup back-to-back LDWEIGHTS
    import concourse.bass_utils as _bu
    _orig_run_command = _bu.run_command

    def _patched_run_command(argv, **kw):
        argv = ["--enable-ldw-opt=true" if a == "--enable-ldw-opt=false"
                else a for a in argv]
        return _orig_run_command(argv, **kw)

    _bu.run_command = _patched_run_command

# ---------------------------------------------------------------------------
# Problem constants
# ---------------------------------------------------------------------------

N_FULL = 2048
B = 4
P = 128
DOUT = 256     # per-path cat chunk width
BW = 128       # bias half width per core
DEC = 128
DINS = (256, 768, 768)   # per-layer input dims
EPS = 1e-12
LEAK = 0.1


# ---------------------------------------------------------------------------
# Program builder
# ---------------------------------------------------------------------------

def build_program(n_cores: int, N: int = N_FULL, stop_phase: int = 99):
    NT = N // P                 # 128-row blocks
    CW = min(512, N)            # psum chunk width
    NI = N // CW                # chunks per full row
    NH = 2 if N >= 1024 else 1  # stage halves (for AG overlap)
    Nh = N // NH
    NCI = NI // NH              # chunks per half

    nc = bacc.Bacc("TRN2", target_bir_lowering=False, debug=False,
                   num_devices=n_cores)

    # --- DRAM I/O (all fp16 except head params / output) ---
    xT_d = nc.dram_tensor("xT", [DINS[0], N], F16, kind="ExternalInput")
    adj_d = nc.dram_tensor("adj", [N, N], F16, kind="ExternalInput")
    invT_d = nc.dram_tensor("invT", [N, N], F16, kind="ExternalInput")
    wa_d = [nc.dram_tensor(f"w{l}a", [N, N], F16, kind="ExternalInput")
            for l in range(3)]
    wc_d = [nc.dram_tensor(f"w{l}c", [DINS[l], DOUT], F16,
                           kind="ExternalInput") for l in range(3)]
    # full bias weights (both halves) -- each core computes the whole bias
    # chunk locally so it never rides the AllGather
    wb_d = [nc.dram_tensor(f"w{l}b", [DINS[l], 2 * BW], F16,
                           kind="ExternalInput") for l in range(3)]
    # p1 is shipped per-core PERMUTED to the core's own feature rows
    # [own 256-block ; own bias 128-block] so the drug-row head projection
    # can run pre-exchange (lrelu commutes with the positive norm scale)
    p1_d = nc.dram_tensor("p1", [3 * P, DEC], FP, kind="ExternalInput")
    p2_d = nc.dram_tensor("p2", [DEC, DEC], FP, kind="ExternalInput")
    # ypred = [raw bilinear; ssq_a; ssq_b]; host divides by the norms
    y_d = nc.dram_tensor("ypred", [3, 1], FP, kind="ExternalOutput")

    groups = [[i, i + 1] for i in range(0, n_cores, 2)]

    with tile.TileContext(nc) as tc:
        with contextlib.ExitStack() as ctx:
            const_p = ctx.enter_context(tc.tile_pool(name="const", bufs=1))
            adjc_p = ctx.enter_context(tc.tile_pool(name="adjc", bufs=1))
            h_p = ctx.enter_context(tc.tile_pool(name="h", bufs=2))
            g_p = ctx.enter_context(tc.tile_pool(name="g", bufs=1))
            a_p = ctx.enter_context(tc.tile_pool(name="a", bufs=1))
            aT_p = ctx.enter_context(tc.tile_pool(name="aT", bufs=1))
            stream_p = ctx.enter_context(tc.tile_pool(name="stream", bufs=3))
            w_p = ctx.enter_context(tc.tile_pool(name="w", bufs=1))
            sq_p = ctx.enter_context(tc.tile_pool(name="sq", bufs=2))
            rn_p = ctx.enter_context(tc.tile_pool(name="rn", bufs=2))
            misc_p = ctx.enter_context(tc.tile_pool(name="misc", bufs=2))
            psum_p = ctx.enter_context(
                tc.tile_pool(name="psum", bufs=8, space="PSUM"))
            dram_p = ctx.enter_context(
                tc.tile_pool(name="dram", bufs=2, space="DRAM"))

            ident16 = const_p.tile([P, P], F16, tag="id16")
            make_identity(nc, ident16)
            ident32 = const_p.tile([P, P], FP, tag="id32")
            make_identity(nc, ident32)
            ones128 = const_p.tile([P, P], BF, tag="o128")
            nc.gpsimd.memset(ones128[:], 1.0)

            D3 = 3 * DOUT

            # copy-engine alternation helper
            _alt = [0]

            def alt_copy(dst, src):
                if _alt[0] % 2 == 0:
                    nc.vector.tensor_copy(dst, src)
                else:
                    nc.scalar.copy(dst, src)
                _alt[0] += 1

            # l0 hT = xT (host-transposed input)
            def h_tiles(nd):
                return [h_p.tile([P, N], F16, tag=f"h{c}", name="hT")
                        for c in range(nd)]

            hT = h_tiles(DINS[0] // P)
            for d, t in enumerate(hT):
                nc.sync.dma_start(t[:], xT_d.ap()[d * P:(d + 1) * P, :])

            # persistent adj cache (fills during l0 a-phase consumption)
            adjc = [adjc_p.tile([P, N], F16, tag=f"adj{k}", name="adjc")
                    for k in range(NT)]
            for k in range(NT):
                nc.sync.dma_start(adjc[k][:], adj_d.ap()[k * P:(k + 1) * P, :])

            invTcols = misc_p.tile([P, NT * 2], F16, tag="ivc", bufs=1)

            # warmup 2-rank collective: absorbs inter-core launch skew while
            # the input streams are still filling, so the layer AllGathers
            # don't pay it
            wu_sb = misc_p.tile([1, 1], FP, tag="wu", bufs=1, name="wu_sb")
            nc.vector.memset(wu_sb[:], 0.0)
            wu_s = dram_p.tile([1, 1], FP, tag="wus", bufs=1, name="wu_s")
            nc.scalar.dma_start(wu_s[:], wu_sb[:])
            wu_g = dram_p.tile([2, 1, 1], FP, tag="wug", bufs=1, name="wu_g")
            nc.gpsimd.collective_compute(
                "AllGather", ALU.bypass, replica_groups=groups,
                ins=[wu_s.opt()], outs=[wu_g.opt()])

            # deferred norm of the previous layer's catT, emitted per half
            # interleaved with this layer's g-phase (fills the AG window)
            pend_norm = None

            for l in range(3):
                din = DINS[l]
                ND = din // P
                last = (l == 2)

                # ---- layer weights ----
                wc_t = w_p.tile([P, ND * DOUT], F16, tag="wc", name="wc_t")
                for d in range(ND):
                    nc.scalar.dma_start(wc_t[:, d * DOUT:(d + 1) * DOUT],
                                        wc_d[l].ap()[d * P:(d + 1) * P, :])
                wb_t = w_p.tile([P, ND * 2 * BW], F16, tag="wb", name="wb_t")
                for d in range(ND):
                    nc.scalar.dma_start(
                        wb_t[:, d * 2 * BW:(d + 1) * 2 * BW],
                        wb_d[l].ap()[d * P:(d + 1) * P, :])

                # ---- g = h @ Wc  [N, 256] node-major; previous layer's
                # deferred norm halves interleave with the g halves ----
                g_t = [g_p.tile([P, DOUT], F16, tag=f"g{it}", name="g_t")
                       for it in range(NT)]
                for gh in range(2):
                    if pend_norm is not None and gh < NH:
                        pend_norm(gh)
                    for it in range(gh * NT // 2, (gh + 1) * NT // 2):
                        pg = psum_p.tile([P, DOUT], FP, tag="ps", name="pg")
                        for d in range(ND):
                            nc.tensor.matmul(
                                pg[:], hT[d][:, it * P:(it + 1) * P],
                                wc_t[:, d * DOUT:(d + 1) * DOUT],
                                start=(d == 0), stop=(d == ND - 1))
                        alt_copy(g_t[it][:], pg[:])

                if not last:
                    hT_next = h_tiles(6)

                    def emit_gb(hT=hT, hT_next=hT_next, wb_t=wb_t, ND=ND):
                        # gb^T = (h @ Wb_full)^T -> c4/c5 locally (bias is
                        # replicated work; it never rides the AllGather)
                        pgb = [[psum_p.tile([P, CW], FP, tag="ps",
                                            name="pgb")
                                for _ in range(NI)] for _ in range(2)]
                        for d in range(ND):
                            for cc in range(2):
                                lhsT = wb_t[:, d * 2 * BW + cc * BW:
                                            d * 2 * BW + (cc + 1) * BW]
                                for ic in range(NI):
                                    nc.tensor.matmul(
                                        pgb[cc][ic][:], lhsT,
                                        hT[d][:, ic * CW:(ic + 1) * CW],
                                        start=(d == 0), stop=(d == ND - 1))
                        for cc in range(2):
                            for ic in range(NI):
                                alt_copy(
                                    hT_next[4 + cc][:, ic * CW:(ic + 1) * CW],
                                    pgb[cc][ic][:])

                    if l == 0:
                        # l0's a-phase is DMA-starved: fill its PE slack
                        emit_gb()

                # ---- a-phase (k-outer): psum = a^T [256, N] ----
                pa = [[psum_p.tile([P, CW], FP, tag="ps", name="pa")
                       for _ in range(NI)] for _ in range(2)]
                for kt in range(NT):
                    wa_t = stream_p.tile([P, N], F16, tag="wa", bufs=4,
                                         name="wa_t")
                    # l0 shares sync with the adj cache fill; keep it on
                    # scalar there, use the quieter sync queue afterwards
                    (nc.scalar if l == 0 else nc.sync).dma_start(
                        wa_t[:], wa_d[l].ap()[kt * P:(kt + 1) * P, :])
                    prod = stream_p.tile([P, N], F16, tag="prod", name="prod")
                    nc.vector.tensor_tensor(prod[:], adjc[kt][:], wa_t[:],
                                            ALU.mult)
                    for mc in range(2):
                        lhsT = g_t[kt][:, mc * P:(mc + 1) * P]
                        for ic in range(NI):
                            nc.tensor.matmul(
                                pa[mc][ic][:], lhsT,
                                prod[:, ic * CW:(ic + 1) * CW],
                                start=(kt == 0), stop=(kt == NT - 1))

                # drain a^T then PE-transpose into a_sb [N, 256] node-major
                aT = [aT_p.tile([P, N], F16, tag=f"aT{mc}", name="aT")
                      for mc in range(2)]
                for mc in range(2):
                    for ic in range(NI):
                        alt_copy(aT[mc][:, ic * CW:(ic + 1) * CW],
                                 pa[mc][ic][:])
                a_sb = [a_p.tile([P, DOUT], F16, tag=f"a{jj}", name="a_sb")
                        for jj in range(NT)]
                for jj in range(NT):
                    for mc in range(2):
                        pt = psum_p.tile([P, P], F16, tag="ps", name="pt")
                        nc.tensor.transpose(pt[:],
                                            aT[mc][:, jj * P:(jj + 1) * P],
                                            ident16[:])
                        alt_copy(a_sb[jj][:, mc * P:(mc + 1) * P], pt[:])

                if not last:
                    # ---- b^T = (inv @ a)^T [256, N], in NH half-width
                    # waves: wave hh streams invT cols [hh*Nh, (hh+1)*Nh)
                    # so wave 0's AllGather hides behind wave 1's matmuls ----
                    ag_q = []
                    for hh in range(NH):
                        pb = [[psum_p.tile([P, CW], FP, tag="ps", name="pb")
                               for _ in range(NCI)] for _ in range(2)]
                        for jj in range(NT):
                            iv = stream_p.tile([P, Nh], F16, tag="inv",
                                               bufs=6, name="iv")
                            (nc.gpsimd if jj % 2 == 0
                             else nc.sync).dma_start(
                                iv[:], invT_d.ap()[jj * P:(jj + 1) * P,
                                                   hh * Nh:(hh + 1) * Nh])
                            if l == 1 and hh == NH - 1:
                                nc.vector.tensor_copy(
                                    invTcols[:, jj * 2:(jj + 1) * 2],
                                    iv[:, Nh - 2:Nh])
                            for cc in range(2):
                                lhsT = a_sb[jj][:, cc * P:(cc + 1) * P]
                                for io in range(NCI):
                                    nc.tensor.matmul(
                                        pb[cc][io][:], lhsT,
                                        iv[:, io * CW:(io + 1) * CW],
                                        start=(jj == 0), stop=(jj == NT - 1))
                        stage_q = dram_p.tile([2 * P, Nh], F16,
                                              tag=f"stq{hh}", name="stage_q")
                        for cc in range(2):
                            for io in range(NCI):
                                stg = misc_p.tile([P, CW], F16, tag="stg",
                                                  bufs=4, name="stg")
                                # vector only: scalar is busy with norm work,
                                # and the AllGather launch gates on these
                                nc.vector.tensor_copy(stg[:], pb[cc][io][:])
                                nc.scalar.dma_start(
                                    stage_q[cc * P:(cc + 1) * P,
                                            io * CW:(io + 1) * CW], stg[:])
                        agt = dram_p.tile([2, 2 * P, Nh], F16, tag=f"agq{hh}",
                                          name="ag_q")
                        nc.gpsimd.collective_compute(
                            "AllGather", ALU.bypass, replica_groups=groups,
                            ins=[stage_q.opt()], outs=[agt.opt()])
                        ag_q.append(agt)

                        if hh == 0 and l > 0:
                            # between the b-waves so the c4/c5 norm inputs
                            # are ready before wave 1's AllGather lands
                            emit_gb()

                    # ---- assemble catT [768, N] (= next hT): readback DMAs
                    # issued now; the norm COMPUTE is deferred into the next
                    # layer's g-phase (closure below) so the PE fills the
                    # AllGather window with useful work ----
                    src = [(0, 0), (0, 1), (1, 0), (1, 1)]
                    for hh in range(NH):
                        for c, (r, rb) in enumerate(src):
                            nc.sync.dma_start(
                                hT_next[c][:, hh * Nh:(hh + 1) * Nh],
                                ag_q[hh][r, rb * P:(rb + 1) * P, :])

                    def pend_norm(hh, hT_next=hT_next):
                        # col norms for half hh: ssq replicated onto all 128
                        # partitions via ones-matrix matmul, then wide
                        # max/sqrt/recip; lrelu BEFORE the scale (rn > 0
                        # commutes with lrelu)
                        for io in range(NCI):
                            ic = hh * NCI + io
                            sl = slice(ic * CW, (ic + 1) * CW)
                            psw = psum_p.tile([P, CW], FP, tag="ps",
                                              name="psw")
                            for c in range(6):
                                sq = sq_p.tile([P, CW], BF, tag="sq",
                                               name="sq")
                                if c % 2 == 0:
                                    nc.vector.tensor_tensor(
                                        sq[:], hT_next[c][:, sl],
                                        hT_next[c][:, sl], ALU.mult)
                                else:
                                    nc.scalar.activation(sq[:],
                                                         hT_next[c][:, sl],
                                                         AF.Square)
                                nc.tensor.matmul(psw[:], ones128[:], sq[:],
                                                 start=(c == 0),
                                                 stop=(c == 5))
                                tmp = sq_p.tile([P, CW], F16, tag="lk",
                                                bufs=3, name="tmp")
                                nc.scalar.mul(tmp[:], hT_next[c][:, sl],
                                              LEAK)
                                nc.vector.tensor_max(hT_next[c][:, sl],
                                                     hT_next[c][:, sl],
                                                     tmp[:])
                            nrm = rn_p.tile([P, CW], FP, tag="nrm",
                                            name="nrm")
                            nc.vector.tensor_scalar_max(nrm[:], psw[:],
                                                        EPS * EPS)
                            nc.scalar.activation(nrm[:], nrm[:], AF.Sqrt)
                            rn = rn_p.tile([P, CW], FP, tag="rn", name="rn")
                            nc.vector.reciprocal_approx_fast(rn[:], nrm[:])
                            for c in range(6):
                                nc.vector.tensor_tensor(hT_next[c][:, sl],
                                                        hT_next[c][:, sl],
                                                        rn[:], ALU.mult)
                    hT = hT_next
                else:
                    # ---- l2: only drug rows N-2, N-1 ----
                    # head params (tiny; needed only here)
                    p1_t = const_p.tile([P, 3 * DEC], FP, tag="p1")
                    for d3 in range(3):
                        nc.scalar.dma_start(
                            p1_t[:, d3 * DEC:(d3 + 1) * DEC],
                            p1_d.ap()[d3 * P:(d3 + 1) * P, :])
                    p2_t = const_p.tile([P, DEC], FP, tag="p2")
                    nc.scalar.dma_start(p2_t[:], p2_d.ap())

                    pb2 = psum_p.tile([2, DOUT], FP, tag="ps", name="pb2")
                    for jj in range(NT):
                        nc.tensor.matmul(pb2[:],
                                         invTcols[:, jj * 2:(jj + 1) * 2],
                                         a_sb[jj][:],
                                         start=(jj == 0), stop=(jj == NT - 1))
                    # own-half bias for the drug rows (host ships w2b with
                    # the core's own half in cols 0:BW)
                    pbias = psum_p.tile([2, BW], FP, tag="ps", name="pbias")
                    for d in range(ND):
                        nc.tensor.matmul(
                            pbias[:], hT[d][:, N - 2:N],
                            wb_t[:, d * 2 * BW:d * 2 * BW + BW],
                            start=(d == 0), stop=(d == ND - 1))
                    # cat_own [2, 384]; ssq BEFORE lrelu (norm uses raw cat)
                    co = misc_p.tile([2, 3 * P], FP, tag="co", bufs=1,
                                     name="co")
                    nc.vector.tensor_copy(co[:, 0:DOUT], pb2[:])
                    nc.scalar.copy(co[:, DOUT:3 * P], pbias[:])
                    csq = misc_p.tile([2, 3 * P], FP, tag="csq", bufs=1,
                                      name="csq")
                    nc.vector.tensor_tensor(csq[:], co[:], co[:], ALU.mult)
                    cssq = misc_p.tile([2, 1], FP, tag="cssq", bufs=1,
                                       name="cssq")
                    nc.vector.tensor_reduce(cssq[:], csq[:], AxisListType.X,
                                            ALU.add)
                    # lrelu in place (positive norm scale commutes)
                    ctmp = misc_p.tile([2, 3 * P], FP, tag="ctmp", bufs=1,
                                       name="ctmp")
                    nc.scalar.mul(ctmp[:], co[:], LEAK)
                    nc.vector.tensor_max(co[:], co[:], ctmp[:])
                    # v_own = P1_own^T @ lrelu(cat_own)^T : [DEC, 2]
                    coT = misc_p.tile([P, 6], FP, tag="coT", bufs=1,
                                      name="coT")
                    for d3 in range(3):
                        pt = psum_p.tile([P, 2], FP, tag="ps", name="pt2")
                        nc.tensor.transpose(
                            pt[:], co[:, d3 * P:(d3 + 1) * P],
                            ident32[0:2, 0:2])
                        nc.vector.tensor_copy(coT[:, d3 * 2:(d3 + 1) * 2],
                                              pt[:])
                    pv = psum_p.tile([P, 2], FP, tag="ps", name="pv")
                    for d3 in range(3):
                        nc.tensor.matmul(pv[:],
                                         p1_t[:, d3 * DEC:(d3 + 1) * DEC],
                                         coT[:, d3 * 2:(d3 + 1) * 2],
                                         start=(d3 == 0), stop=(d3 == 2))
                    # stage [DEC, 3] = [v_own | ssq_own in rows 0:2 of col 2]
                    s3 = misc_p.tile([DEC, 3], FP, tag="s3", bufs=1,
                                     name="s3")
                    nc.vector.memset(s3[:, 2:3], 0.0)
                    nc.vector.tensor_copy(s3[:, 0:2], pv[:])
                    nc.scalar.copy(s3[0:2, 2:3], cssq[:])
                    stage3 = dram_p.tile([DEC, 3], FP, tag="stl2", bufs=1,
                                         name="stage3")
                    nc.scalar.dma_start(stage3[:], s3[:])
                    ag3 = dram_p.tile([2, DEC, 3], FP, tag="ag3", bufs=1,
                                      name="ag3")
                    nc.gpsimd.collective_compute(
                        "AllGather", ALU.bypass, replica_groups=groups,
                        ins=[stage3.opt()], outs=[ag3.opt()])
                    va = misc_p.tile([DEC, 3], FP, tag="va", bufs=1,
                                     name="va")
                    vb = misc_p.tile([DEC, 3], FP, tag="vb", bufs=1,
                                     name="vb")
                    nc.sync.dma_start(va[:], ag3[0, :, :])
                    nc.sync.dma_start(vb[:], ag3[1, :, :])
                    v_sb = misc_p.tile([DEC, 3], FP, tag="v_sb", bufs=1,
                                       name="v_sb")
                    nc.vector.tensor_tensor(v_sb[:], va[:], vb[:], ALU.add)

            # ---- head: raw = (v0)^T P2 v1; ssqs shipped to host ----
            pw = psum_p.tile([P, 1], FP, tag="ps", name="pw")
            nc.tensor.matmul(pw[:], p2_t[:], v_sb[:, 0:1], start=True,
                             stop=True)
            t_sb = misc_p.tile([P, 1], FP, tag="t_sb", bufs=1)
            nc.vector.tensor_copy(t_sb[:], pw[:])
            py = psum_p.tile([1, 1], FP, tag="ps", name="py")
            nc.tensor.matmul(py[:], t_sb[:], v_sb[:, 1:2], start=True,
                             stop=True)
            y_sb = misc_p.tile([1, 1], FP, tag="y_sb", bufs=1)
            nc.vector.tensor_copy(y_sb[:], py[:])
            nc.sync.dma_start(y_d.ap()[0:1, :], y_sb[:])
            nc.sync.dma_start(y_d.ap()[1:3, :], v_sb[0:2, 2:3])

    nc.compile()
    return nc


# ---------------------------------------------------------------------------
# Host-side input prep
# ---------------------------------------------------------------------------

def make_in_maps(inputs: dict, n_cores: int):
    """Per-core input dicts. Core 2b = up path of batch b, 2b+1 = down."""
    f32 = lambda a: np.ascontiguousarray(np.asarray(a, dtype=np.float32))
    f16 = lambda a: np.ascontiguousarray(
        np.asarray(a, dtype=np.float32).astype(np.float16))

    def bake(w):
        w = np.array(w, dtype=np.float32, copy=True)
        w[-2:, :] = 1.0
        w[:, -2:] = 1.0
        return w

    p1 = np.asarray(inputs["parameter1"], dtype=np.float32)
    maps = []
    for c in range(n_cores):
        b, down = divmod(c, 2)
        # p1 rows permuted to this core's local drug-feature order:
        # [own 256-chunk ; own bias 128-half]
        if not down:
            p1c = np.concatenate([p1[0:256], p1[512:640]], axis=0)
        else:
            p1c = np.concatenate([p1[256:512], p1[640:768]], axis=0)
        m = {
            "xT": f16(np.asarray(inputs["x"][b]).T),
            "p1": f32(p1c),
            "p2": f32(inputs["parameter2"]),
        }
        for l in range(2):
            m[f"w{l}b"] = f16(inputs[f"l{l}_bias"])  # full, both cores
        # l2 bias: own half first (only the own half is used)
        b2 = np.asarray(inputs["l2_bias"], dtype=np.float32)
        if not down:
            m["w2b"] = f16(np.concatenate([b2[:, :BW], b2[:, BW:]], axis=1))
        else:
            m["w2b"] = f16(np.concatenate([b2[:, BW:], b2[:, :BW]], axis=1))
        if not down:
            m["adj"] = f16(inputs["adj"][b])
            m["invT"] = f16(np.asarray(inputs["up_inv_deg"][b]).T)
            for l in range(3):
                m[f"w{l}a"] = f16(bake(inputs[f"l{l}_up_adj_w"]))
                m[f"w{l}c"] = f16(inputs[f"l{l}_up_w"])
        else:
            m["adj"] = f16(np.asarray(inputs["adj"][b]).T)
            m["invT"] = f16(np.asarray(inputs["down_inv_deg"][b]).T)
            for l in range(3):
                m[f"w{l}a"] = f16(bake(inputs[f"l{l}_down_adj_w"]).T)
                m[f"w{l}c"] = f16(inputs[f"l{l}_down_w"])
        maps.append(m)
    return maps


_nc_cache = {}


def _get_program(n_cores, N):
    key = (n_cores, N)
    if key not in _nc_cache:
        _nc_cache[key] = build_program(n_cores, N)
    return _nc_cache[key]


def kernel(**inputs) -> np.ndarray:
    n_cores = 8
    nc = _get_program(n_cores, N_FULL)
    in_maps = make_in_maps(inputs, n_cores)
    res = run_bass_kernel_spmd(nc, in_maps, core_ids=list(range(n_cores)))
    out = np.zeros((B, 1), dtype=np.float32)
    for b in range(B):
        raw, ssq_a, ssq_b = res.results[2 * b]["ypred"][:, 0]
        out[b, 0] = raw / (max(np.sqrt(ssq_a), EPS) * max(np.sqrt(ssq_b), EPS))
    return out


# revision 56
# speedup vs baseline: 1.0610x; 1.0610x over previous
"""BiGraphSAGEDecoder Trainium2 kernel (v2: fp16 + reassociated matmul chain).

Sharding: 8 cores = 4 batches x {up-path, down-path}. One SPMD bass program;
the up/down asymmetry is handled purely by data (down cores receive host-
transposed adjacency / adjacency-weight / inv-degree matrices).

Key restructurings vs the reference math:
  * associativity:  inv @ (prod^T @ h) @ W  ->  inv @ (prod^T @ (h @ W)),
    so both N x N matmuls run at width 256 instead of width din (768).
  * all feature maps are kept FEATURE-MAJOR (hT: [feature part, node free]);
    column L2 norms via a ones-vector matmul on the PE. No hidden-state
    transposes anywhere.
  * layer 2 computes only the two drug rows after a2 = prod^T @ g2
    (b restricted to 2 rows of inv).
  * fp16 storage end-to-end (PSUM accumulation fp32); host pre-converts.

Per layer (per core, its path), with prod = adj . Wadj' (mask baked on host):
  g    = h @ Wc                      (PE, lhsT = hT tiles)          [N, 256]
  a^T  = (prod^T @ g)^T              (PE, k-outer, 8 psum banks)    [256, N]
  a_sb = transpose(a^T)              (PE transposes)                [N, 256]
  b^T  = (inv @ a)^T                 (PE, lhsT = a_sb, rhs = invT)  [256, N]
  gb^T = (h @ Wb_half)^T             (PE, lhsT = Wb, rhs = hT)      [128, N]
  stage [384, N] = [b^T ; gb^T] -> 2-rank AllGather -> catT [768, N]
  hT'  = lrelu(catT / max(||catT||_col, eps))   (DVE + ones-matmul norms)
Head: bilinear form on PE in fp32 (tiny).
"""

import os
import sys
import types
import contextlib

sys.path.insert(0, "/opt/trn_rl_repo")

import numpy as np

import concourse.bass as bass
import concourse.tile as tile
from concourse import mybir, bacc
from concourse.mybir import AxisListType
from concourse.masks import make_identity
from concourse.bass_utils import run_bass_kernel_spmd

FP = mybir.dt.float32
F16 = mybir.dt.float16
BF = mybir.dt.bfloat16
AF = mybir.ActivationFunctionType
ALU = mybir.AluOpType

# ---------------------------------------------------------------------------
# Environment patches (required for this container's toolchain)
# ---------------------------------------------------------------------------


def install_ntff_shim():
    """antenv.axon_hooks is absent in this image; provide it so trace=True
    profiling works (used by test.py, harmless otherwise)."""
    try:
        import antenv.axon_hooks  # noqa: F401
        return
    except ImportError:
        pass
    try:
        import antenv
    except ImportError:
        return
    mod = types.ModuleType("antenv.axon_hooks")
    _holder = {"hook": None}
    mod.set_axon_ntff_profile_hook = lambda h: _holder.__setitem__("hook", h)
    mod.get_axon_ntff_profile_hook = lambda: _holder["hook"]
    sys.modules["antenv.axon_hooks"] = mod
    antenv.axon_hooks = mod
    try:
        from trn_agent_boot.trn_boot import _ntff_profile_via_ctypes

        hook = _ntff_profile_via_ctypes("/opt/axon/libaxon_pjrt.so")
        if hook is not None:
            mod.set_axon_ntff_profile_hook(hook)
    except Exception:
        pass


install_ntff_shim()

if os.environ.get("KGSD_LDW_OPT", "0") != "0":
    # let walrus dedup back-to-back LDWEIGHTS
    import concourse.bass_utils as _bu
    _orig_run_command = _bu.run_command

    def _patched_run_command(argv, **kw):
        argv = ["--enable-ldw-opt=true" if a == "--enable-ldw-opt=false"
                else a for a in argv]
        return _orig_run_command(argv, **kw)

    _bu.run_command = _patched_run_command

# ---------------------------------------------------------------------------
# Problem constants
# ---------------------------------------------------------------------------

N_FULL = 2048
B = 4
P = 128
DOUT = 256     # per-path cat chunk width
BW = 128       # bias half width per core
DEC = 128
DINS = (256, 768, 768)   # per-layer input dims
EPS = 1e-12
LEAK = 0.1


# ---------------------------------------------------------------------------
# Program builder
# ---------------------------------------------------------------------------

def build_program(n_cores: int, N: int = N_FULL, stop_phase: int = 99):
    NT = N // P                 # 128-row blocks
    CW = min(512, N)            # psum chunk width
    NI = N // CW                # chunks per full row
    NH = 2 if N >= 1024 else 1  # stage halves (for AG overlap)
    Nh = N // NH
    NCI = NI // NH              # chunks per half

    nc = bacc.Bacc("TRN2", target_bir_lowering=False, debug=False,
                   num_devices=n_cores)

    # --- DRAM I/O (all fp16 except head params / output) ---
    xT_d = nc.dram_tensor("xT", [DINS[0], N], F16, kind="ExternalInput")
    adj_d = nc.dram_tensor("adj", [N, N], F16, kind="ExternalInput")
    invT_d = nc.dram_tensor("invT", [N, N], F16, kind="ExternalInput")
    wa_d = [nc.dram_tensor(f"w{l}a", [N, N], F16, kind="ExternalInput")
            for l in range(3)]
    wc_d = [nc.dram_tensor(f"w{l}c", [DINS[l], DOUT], F16,
                           kind="ExternalInput") for l in range(3)]
    # full bias weights (both halves) -- each core computes the whole bias
    # chunk locally so it never rides the AllGather
    wb_d = [nc.dram_tensor(f"w{l}b", [DINS[l], 2 * BW], F16,
                           kind="ExternalInput") for l in range(3)]
    # p1 is shipped per-core PERMUTED to the core's own feature rows
    # [own 256-block ; own bias 128-block] so the drug-row head projection
    # can run pre-exchange (lrelu commutes with the positive norm scale)
    p1_d = nc.dram_tensor("p1", [3 * P, DEC], FP, kind="ExternalInput")
    p2_d = nc.dram_tensor("p2", [DEC, DEC], FP, kind="ExternalInput")
    # ypred = [raw bilinear; ssq_a; ssq_b]; host divides by the norms
    y_d = nc.dram_tensor("ypred", [3, 1], FP, kind="ExternalOutput")

    groups = [[i, i + 1] for i in range(0, n_cores, 2)]

    with tile.TileContext(nc) as tc:
        with contextlib.ExitStack() as ctx:
            const_p = ctx.enter_context(tc.tile_pool(name="const", bufs=1))
            adjc_p = ctx.enter_context(tc.tile_pool(name="adjc", bufs=1))
            h_p = ctx.enter_context(tc.tile_pool(name="h", bufs=2))
            g_p = ctx.enter_context(tc.tile_pool(name="g", bufs=1))
            a_p = ctx.enter_context(tc.tile_pool(name="a", bufs=1))
            aT_p = ctx.enter_context(tc.tile_pool(name="aT", bufs=1))
            stream_p = ctx.enter_context(tc.tile_pool(name="stream", bufs=3))
            w_p = ctx.enter_context(tc.tile_pool(name="w", bufs=1))
            sq_p = ctx.enter_context(tc.tile_pool(name="sq", bufs=2))
            rn_p = ctx.enter_context(tc.tile_pool(name="rn", bufs=2))
            misc_p = ctx.enter_context(tc.tile_pool(name="misc", bufs=2))
            psum_p = ctx.enter_context(
                tc.tile_pool(name="psum", bufs=8, space="PSUM"))
            dram_p = ctx.enter_context(
                tc.tile_pool(name="dram", bufs=2, space="DRAM"))

            ident16 = const_p.tile([P, P], F16, tag="id16")
            make_identity(nc, ident16)
            ident32 = const_p.tile([P, P], FP, tag="id32")
            make_identity(nc, ident32)
            ones128 = const_p.tile([P, P], BF, tag="o128")
            nc.gpsimd.memset(ones128[:], 1.0)

            D3 = 3 * DOUT

            # copy-engine alternation helper
            _alt = [0]

            def alt_copy(dst, src):
                if _alt[0] % 2 == 0:
                    nc.vector.tensor_copy(dst, src)
                else:
                    nc.scalar.copy(dst, src)
                _alt[0] += 1

            # l0 hT = xT (host-transposed input)
            def h_tiles(nd):
                return [h_p.tile([P, N], F16, tag=f"h{c}", name="hT")
                        for c in range(nd)]

            hT = h_tiles(DINS[0] // P)
            for d, t in enumerate(hT):
                nc.sync.dma_start(t[:], xT_d.ap()[d * P:(d + 1) * P, :])

            # persistent adj cache (fills during l0 a-phase consumption)
            adjc = [adjc_p.tile([P, N], F16, tag=f"adj{k}", name="adjc")
                    for k in range(NT)]
            for k in range(NT):
                nc.sync.dma_start(adjc[k][:], adj_d.ap()[k * P:(k + 1) * P, :])

            invTcols = misc_p.tile([P, NT * 2], F16, tag="ivc", bufs=1)

            # warmup 2-rank collective: absorbs inter-core launch skew while
            # the input streams are still filling, so the layer AllGathers
            # don't pay it
            wu_sb = misc_p.tile([1, 1], FP, tag="wu", bufs=1, name="wu_sb")
            nc.vector.memset(wu_sb[:], 0.0)
            wu_s = dram_p.tile([1, 1], FP, tag="wus", bufs=1, name="wu_s")
            nc.scalar.dma_start(wu_s[:], wu_sb[:])
            wu_g = dram_p.tile([2, 1, 1], FP, tag="wug", bufs=1, name="wu_g")
            nc.gpsimd.collective_compute(
                "AllGather", ALU.bypass, replica_groups=groups,
                ins=[wu_s.opt()], outs=[wu_g.opt()])

            # deferred norm of the previous layer's catT, emitted per half
            # interleaved with this layer's g-phase (fills the AG window)
            pend_norm = None

            for l in range(3):
                din = DINS[l]
                ND = din // P
                last = (l == 2)

                # ---- layer weights ----
                wc_t = w_p.tile([P, ND * DOUT], F16, tag="wc", name="wc_t")
                for d in range(ND):
                    nc.scalar.dma_start(wc_t[:, d * DOUT:(d + 1) * DOUT],
                                        wc_d[l].ap()[d * P:(d + 1) * P, :])
                wb_t = w_p.tile([P, ND * 2 * BW], F16, tag="wb", name="wb_t")
                for d in range(ND):
                    nc.scalar.dma_start(
                        wb_t[:, d * 2 * BW:(d + 1) * 2 * BW],
                        wb_d[l].ap()[d * P:(d + 1) * P, :])

                # ---- g = h @ Wc  [N, 256] node-major; previous layer's
                # deferred norm halves interleave with the g halves ----
                g_t = [g_p.tile([P, DOUT], F16, tag=f"g{it}", name="g_t")
                       for it in range(NT)]
                for gh in range(2):
                    if pend_norm is not None and gh < NH:
                        pend_norm(gh)
                    for it in range(gh * NT // 2, (gh + 1) * NT // 2):
                        pg = psum_p.tile([P, DOUT], FP, tag="ps", name="pg")
                        for d in range(ND):
                            nc.tensor.matmul(
                                pg[:], hT[d][:, it * P:(it + 1) * P],
                                wc_t[:, d * DOUT:(d + 1) * DOUT],
                                start=(d == 0), stop=(d == ND - 1))
                        alt_copy(g_t[it][:], pg[:])

                if not last:
                    hT_next = h_tiles(6)

                    def emit_gb(hT=hT, hT_next=hT_next, wb_t=wb_t, ND=ND):
                        # gb^T = (h @ Wb_full)^T -> c4/c5 locally (bias is
                        # replicated work; it never rides the AllGather)
                        pgb = [[psum_p.tile([P, CW], FP, tag="ps",
                                            name="pgb")
                                for _ in range(NI)] for _ in range(2)]
                        for d in range(ND):
                            for cc in range(2):
                                lhsT = wb_t[:, d * 2 * BW + cc * BW:
                                            d * 2 * BW + (cc + 1) * BW]
                                for ic in range(NI):
                                    nc.tensor.matmul(
                                        pgb[cc][ic][:], lhsT,
                                        hT[d][:, ic * CW:(ic + 1) * CW],
                                        start=(d == 0), stop=(d == ND - 1))
                        for cc in range(2):
                            for ic in range(NI):
                                alt_copy(
                                    hT_next[4 + cc][:, ic * CW:(ic + 1) * CW],
                                    pgb[cc][ic][:])

                    if l == 0:
                        # l0's a-phase is DMA-starved: fill its PE slack
                        emit_gb()

                # ---- a-phase (k-outer): psum = a^T [256, N] ----
                pa = [[psum_p.tile([P, CW], FP, tag="ps", name="pa")
                       for _ in range(NI)] for _ in range(2)]
                for kt in range(NT):
                    wa_t = stream_p.tile([P, N], F16, tag="wa", bufs=4,
                                         name="wa_t")
                    # l0 shares sync with the adj cache fill; keep it on
                    # scalar there, use the quieter sync queue afterwards
                    (nc.scalar if l == 0 else nc.sync).dma_start(
                        wa_t[:], wa_d[l].ap()[kt * P:(kt + 1) * P, :])
                    prod = stream_p.tile([P, N], F16, tag="prod", name="prod")
                    nc.vector.tensor_tensor(prod[:], adjc[kt][:], wa_t[:],
                                            ALU.mult)
                    for mc in range(2):
                        lhsT = g_t[kt][:, mc * P:(mc + 1) * P]
                        for ic in range(NI):
                            nc.tensor.matmul(
                                pa[mc][ic][:], lhsT,
                                prod[:, ic * CW:(ic + 1) * CW],
                                start=(kt == 0), stop=(kt == NT - 1))

                # drain a^T then PE-transpose into a_sb [N, 256] node-major
                aT = [aT_p.tile([P, N], F16, tag=f"aT{mc}", name="aT")
                      for mc in range(2)]
                for mc in range(2):
                    for ic in range(NI):
                        alt_copy(aT[mc][:, ic * CW:(ic + 1) * CW],
                                 pa[mc][ic][:])
                a_sb = [a_p.tile([P, DOUT], F16, tag=f"a{jj}", name="a_sb")
                        for jj in range(NT)]
                for jj in range(NT):
                    for mc in range(2):
                        pt = psum_p.tile([P, P], F16, tag="ps", name="pt")
                        nc.tensor.transpose(pt[:],
                                            aT[mc][:, jj * P:(jj + 1) * P],
                                            ident16[:])
                        alt_copy(a_sb[jj][:, mc * P:(mc + 1) * P], pt[:])

                if not last:
                    # ---- b^T = (inv @ a)^T [256, N], in NH half-width
                    # waves: wave hh streams invT cols [hh*Nh, (hh+1)*Nh)
                    # so wave 0's AllGather hides behind wave 1's matmuls ----
                    ag_q = []
                    for hh in range(NH):
                        pb = [[psum_p.tile([P, CW], FP, tag="ps", name="pb")
                               for _ in range(NCI)] for _ in range(2)]
                        for jj in range(NT):
                            iv = stream_p.tile([P, Nh], F16, tag="inv",
                                               bufs=6, name="iv")
                            (nc.gpsimd if jj % 2 == 0
                             else nc.sync).dma_start(
                                iv[:], invT_d.ap()[jj * P:(jj + 1) * P,
                                                   hh * Nh:(hh + 1) * Nh])
                            if l == 1 and hh == NH - 1:
                                nc.vector.tensor_copy(
                                    invTcols[:, jj * 2:(jj + 1) * 2],
                                    iv[:, Nh - 2:Nh])
                            for cc in range(2):
                                lhsT = a_sb[jj][:, cc * P:(cc + 1) * P]
                                for io in range(NCI):
                                    nc.tensor.matmul(
                                        pb[cc][io][:], lhsT,
                                        iv[:, io * CW:(io + 1) * CW],
                                        start=(jj == 0), stop=(jj == NT - 1))
                        stage_q = dram_p.tile([2 * P, Nh], F16,
                                              tag=f"stq{hh}", name="stage_q")
                        for cc in range(2):
                            for io in range(NCI):
                                stg = misc_p.tile([P, CW], F16, tag="stg",
                                                  bufs=4, name="stg")
                                alt_copy(stg[:], pb[cc][io][:])
                                nc.scalar.dma_start(
                                    stage_q[cc * P:(cc + 1) * P,
                                            io * CW:(io + 1) * CW], stg[:])
                        agt = dram_p.tile([2, 2 * P, Nh], F16, tag=f"agq{hh}",
                                          name="ag_q")
                        nc.gpsimd.collective_compute(
                            "AllGather", ALU.bypass, replica_groups=groups,
                            ins=[stage_q.opt()], outs=[agt.opt()])
                        ag_q.append(agt)

                        if hh == 0 and l > 0:
                            # between the b-waves so the c4/c5 norm inputs
                            # are ready before wave 1's AllGather lands
                            emit_gb()

                    # ---- assemble catT [768, N] (= next hT): readback DMAs
                    # issued now; the norm COMPUTE is deferred into the next
                    # layer's g-phase (closure below) so the PE fills the
                    # AllGather window with useful work ----
                    src = [(0, 0), (0, 1), (1, 0), (1, 1)]
                    for hh in range(NH):
                        for c, (r, rb) in enumerate(src):
                            nc.sync.dma_start(
                                hT_next[c][:, hh * Nh:(hh + 1) * Nh],
                                ag_q[hh][r, rb * P:(rb + 1) * P, :])

                    def pend_norm(hh, hT_next=hT_next):
                        # col norms for half hh: ssq replicated onto all 128
                        # partitions via ones-matrix matmul, then wide
                        # max/sqrt/recip; lrelu BEFORE the scale (rn > 0
                        # commutes with lrelu)
                        for io in range(NCI):
                            ic = hh * NCI + io
                            sl = slice(ic * CW, (ic + 1) * CW)
                            psw = psum_p.tile([P, CW], FP, tag="ps",
                                              name="psw")
                            for c in range(6):
                                sq = sq_p.tile([P, CW], BF, tag="sq",
                                               name="sq")
                                if c % 2 == 0:
                                    nc.vector.tensor_tensor(
                                        sq[:], hT_next[c][:, sl],
                                        hT_next[c][:, sl], ALU.mult)
                                else:
                                    nc.scalar.activation(sq[:],
                                                         hT_next[c][:, sl],
                                                         AF.Square)
                                nc.tensor.matmul(psw[:], ones128[:], sq[:],
                                                 start=(c == 0),
                                                 stop=(c == 5))
                                tmp = sq_p.tile([P, CW], F16, tag="lk",
                                                bufs=3, name="tmp")
                                nc.scalar.mul(tmp[:], hT_next[c][:, sl],
                                              LEAK)
                                nc.vector.tensor_max(hT_next[c][:, sl],
                                                     hT_next[c][:, sl],
                                                     tmp[:])
                            nrm = rn_p.tile([P, CW], FP, tag="nrm",
                                            name="nrm")
                            nc.vector.tensor_scalar_max(nrm[:], psw[:],
                                                        EPS * EPS)
                            nc.scalar.activation(nrm[:], nrm[:], AF.Sqrt)
                            rn = rn_p.tile([P, CW], FP, tag="rn", name="rn")
                            nc.vector.reciprocal_approx_fast(rn[:], nrm[:])
                            for c in range(6):
                                nc.vector.tensor_tensor(hT_next[c][:, sl],
                                                        hT_next[c][:, sl],
                                                        rn[:], ALU.mult)
                    hT = hT_next
                else:
                    # ---- l2: only drug rows N-2, N-1 ----
                    # head params (tiny; needed only here)
                    p1_t = const_p.tile([P, 3 * DEC], FP, tag="p1")
                    for d3 in range(3):
                        nc.scalar.dma_start(
                            p1_t[:, d3 * DEC:(d3 + 1) * DEC],
                            p1_d.ap()[d3 * P:(d3 + 1) * P, :])
                    p2_t = const_p.tile([P, DEC], FP, tag="p2")
                    nc.scalar.dma_start(p2_t[:], p2_d.ap())

                    pb2 = psum_p.tile([2, DOUT], FP, tag="ps", name="pb2")
                    for jj in range(NT):
                        nc.tensor.matmul(pb2[:],
                                         invTcols[:, jj * 2:(jj + 1) * 2],
                                         a_sb[jj][:],
                                         start=(jj == 0), stop=(jj == NT - 1))
                    # own-half bias for the drug rows (host ships w2b with
                    # the core's own half in cols 0:BW)
                    pbias = psum_p.tile([2, BW], FP, tag="ps", name="pbias")
                    for d in range(ND):
                        nc.tensor.matmul(
                            pbias[:], hT[d][:, N - 2:N],
                            wb_t[:, d * 2 * BW:d * 2 * BW + BW],
                            start=(d == 0), stop=(d == ND - 1))
                    # cat_own [2, 384]; ssq BEFORE lrelu (norm uses raw cat)
                    co = misc_p.tile([2, 3 * P], FP, tag="co", bufs=1,
                                     name="co")
                    nc.vector.tensor_copy(co[:, 0:DOUT], pb2[:])
                    nc.scalar.copy(co[:, DOUT:3 * P], pbias[:])
                    csq = misc_p.tile([2, 3 * P], FP, tag="csq", bufs=1,
                                      name="csq")
                    nc.vector.tensor_tensor(csq[:], co[:], co[:], ALU.mult)
                    cssq = misc_p.tile([2, 1], FP, tag="cssq", bufs=1,
                                       name="cssq")
                    nc.vector.tensor_reduce(cssq[:], csq[:], AxisListType.X,
                                            ALU.add)
                    # lrelu in place (positive norm scale commutes)
                    ctmp = misc_p.tile([2, 3 * P], FP, tag="ctmp", bufs=1,
                                       name="ctmp")
                    nc.scalar.mul(ctmp[:], co[:], LEAK)
                    nc.vector.tensor_max(co[:], co[:], ctmp[:])
                    # v_own = P1_own^T @ lrelu(cat_own)^T : [DEC, 2]
                    coT = misc_p.tile([P, 6], FP, tag="coT", bufs=1,
                                      name="coT")
                    for d3 in range(3):
                        pt = psum_p.tile([P, 2], FP, tag="ps", name="pt2")
                        nc.tensor.transpose(
                            pt[:], co[:, d3 * P:(d3 + 1) * P],
                            ident32[0:2, 0:2])
                        nc.vector.tensor_copy(coT[:, d3 * 2:(d3 + 1) * 2],
                                              pt[:])
                    pv = psum_p.tile([P, 2], FP, tag="ps", name="pv")
                    for d3 in range(3):
                        nc.tensor.matmul(pv[:],
                                         p1_t[:, d3 * DEC:(d3 + 1) * DEC],
                                         coT[:, d3 * 2:(d3 + 1) * 2],
                                         start=(d3 == 0), stop=(d3 == 2))
                    # stage [DEC, 3] = [v_own | ssq_own in rows 0:2 of col 2]
                    s3 = misc_p.tile([DEC, 3], FP, tag="s3", bufs=1,
                                     name="s3")
                    nc.vector.memset(s3[:, 2:3], 0.0)
                    nc.vector.tensor_copy(s3[:, 0:2], pv[:])
                    nc.scalar.copy(s3[0:2, 2:3], cssq[:])
                    stage3 = dram_p.tile([DEC, 3], FP, tag="stl2", bufs=1,
                                         name="stage3")
                    nc.scalar.dma_start(stage3[:], s3[:])
                    ag3 = dram_p.tile([2, DEC, 3], FP, tag="ag3", bufs=1,
                                      name="ag3")
                    nc.gpsimd.collective_compute(
                        "AllGather", ALU.bypass, replica_groups=groups,
                        ins=[stage3.opt()], outs=[ag3.opt()])
                    va = misc_p.tile([DEC, 3], FP, tag="va", bufs=1,
                                     name="va")
                    vb = misc_p.tile([DEC, 3], FP, tag="vb", bufs=1,
                                     name="vb")
                    nc.sync.dma_start(va[:], ag3[0, :, :])
                    nc.sync.dma_start(vb[:], ag3[1, :, :])
                    v_sb = misc_p.tile([DEC, 3], FP, tag="v_sb", bufs=1,
                                       name="v_sb")
                    nc.vector.tensor_tensor(v_sb[:], va[:], vb[:], ALU.add)

            # ---- head: raw = (v0)^T P2 v1; ssqs shipped to host ----
            pw = psum_p.tile([P, 1], FP, tag="ps", name="pw")
            nc.tensor.matmul(pw[:], p2_t[:], v_sb[:, 0:1], start=True,
                             stop=True)
            t_sb = misc_p.tile([P, 1], FP, tag="t_sb", bufs=1)
            nc.vector.tensor_copy(t_sb[:], pw[:])
            py = psum_p.tile([1, 1], FP, tag="ps", name="py")
            nc.tensor.matmul(py[:], t_sb[:], v_sb[:, 1:2], start=True,
                             stop=True)
            y_sb = misc_p.tile([1, 1], FP, tag="y_sb", bufs=1)
            nc.vector.tensor_copy(y_sb[:], py[:])
            nc.sync.dma_start(y_d.ap()[0:1, :], y_sb[:])
            nc.sync.dma_start(y_d.ap()[1:3, :], v_sb[0:2, 2:3])

    nc.compile()
    return nc


# ---------------------------------------------------------------------------
# Host-side input prep
# ---------------------------------------------------------------------------

def make_in_maps(inputs: dict, n_cores: int):
    """Per-core input dicts. Core 2b = up path of batch b, 2b+1 = down."""
    f32 = lambda a: np.ascontiguousarray(np.asarray(a, dtype=np.float32))
    f16 = lambda a: np.ascontiguousarray(
        np.asarray(a, dtype=np.float32).astype(np.float16))

    def bake(w):
        w = np.array(w, dtype=np.float32, copy=True)
        w[-2:, :] = 1.0
        w[:, -2:] = 1.0
        return w

    p1 = np.asarray(inputs["parameter1"], dtype=np.float32)
    maps = []
    for c in range(n_cores):
        b, down = divmod(c, 2)
        # p1 rows permuted to this core's local drug-feature order:
        # [own 256-chunk ; own bias 128-half]
        if not down:
            p1c = np.concatenate([p1[0:256], p1[512:640]], axis=0)
        else:
            p1c = np.concatenate([p1[256:512], p1[640:768]], axis=0)
        m = {
            "xT": f16(np.asarray(inputs["x"][b]).T),
            "p1": f32(p1c),
            "p2": f32(inputs["parameter2"]),
        }
        for l in range(2):
            m[f"w{l}b"] = f16(inputs[f"l{l}_bias"])  # full, both cores
        # l2 bias: own half first (only the own half is used)
        b2 = np.asarray(inputs["l2_bias"], dtype=np.float32)
        if not down:
            m["w2b"] = f16(np.concatenate([b2[:, :BW], b2[:, BW:]], axis=1))
        else:
            m["w2b"] = f16(np.concatenate([b2[:, BW:], b2[:, :BW]], axis=1))
        if not down:
            m["adj"] = f16(inputs["adj"][b])
            m["invT"] = f16(np.asarray(inputs["up_inv_deg"][b]).T)
            for l in range(3):
                m[f"w{l}a"] = f16(bake(inputs[f"l{l}_up_adj_w"]))
                m[f"w{l}c"] = f16(inputs[f"l{l}_up_w"])
        else:
            m["adj"] = f16(np.asarray(inputs["adj"][b]).T)
            m["invT"] = f16(np.asarray(inputs["down_inv_deg"][b]).T)
            for l in range(3):
                m[f"w{l}a"] = f16(bake(inputs[f"l{l}_down_adj_w"]).T)
                m[f"w{l}c"] = f16(inputs[f"l{l}_down_w"])
        maps.append(m)
    return maps


_nc_cache = {}


def _get_program(n_cores, N):
    key = (n_cores, N)
    if key not in _nc_cache:
        _nc_cache[key] = build_program(n_cores, N)
    return _nc_cache[key]


def kernel(**inputs) -> np.ndarray:
    n_cores = 8
    nc = _get_program(n_cores, N_FULL)
    in_maps = make_in_maps(inputs, n_cores)
    res = run_bass_kernel_spmd(nc, in_maps, core_ids=list(range(n_cores)))
    out = np.zeros((B, 1), dtype=np.float32)
    for b in range(B):
        raw, ssq_a, ssq_b = res.results[2 * b]["ypred"][:, 0]
        out[b, 0] = raw / (max(np.sqrt(ssq_a), EPS) * max(np.sqrt(ssq_b), EPS))
    return out
